# revision 15
# baseline (speedup 1.0000x reference)
"""Tensor-parallel MiniGPT single-token decode step on 8 Trainium2 NeuronCores.

Sharding (per core i of 8):
  - attention: heads 2i, 2i+1 (head_dim 128 -> cols i*256:(i+1)*256 of E=2048);
    wq/wk/wv row-sharded, wo column-sharded, KV cache column-sharded by head.
  - MLP: w1 row-sharded (1024 rows/core), w2 column-sharded.
  - LM head: vocab-sharded (50257 padded to 8*6283=50264 rows).
  - Two 8KB AllReduces combine the wo- and w2- partial sums; logits are
    gathered on the host.

All weights are cast to bf16 on the host and laid out as [128, F]
partition-major arrays, so every device DMA is one contiguous ~1-1.6MB run at
half the fp32 byte count. Activations stay fp32; PE matmuls take bf16 operands
and accumulate fp32 in PSUM.

Matvec strategy: PE does nearly everything. x-chunks are the [128, 1]
stationary operand, weight tiles stream as the moving operand in N<=512 chunks
with PSUM accumulation across k-blocks. Output chunks are spread across PSUM
*partition rows* 0/32/64/96 (PE column tiling via out.base_partition), so up
to 4 chains run concurrently in different column groups of the array and 4
chains share one PSUM bank. Attention scores are computed directly in column
layout (key-block stationary, q moving); exp runs on ACT straight from PSUM
with accum_out providing softmax denominators. Row->column transposes use K=1
matmuls (lhsT = the row, rhs = a [1,1] one) instead of DMA reshapes. The
collective path is warmed with an early AllReduce whose result is consumed at
the output tail.
"""

import numpy as np
import ml_dtypes

N_CORES = 8
E = 2048
HPC = 2  # heads per core
EPC = HPC * 128  # 256
T = 8192
VOCAB = 50257
VPC = 6283  # padded vocab rows per core (8 * 6283 = 50264)
SCALE = float(1.0 / np.sqrt(128.0))
EPS = 1e-5

_CACHE = {}
TRACE = False


def _build_nc():
    import concourse.bacc as bacc
    import concourse.mybir as mybir
    import concourse.tile as tile

    AF = mybir.ActivationFunctionType
    MUL = mybir.AluOpType.mult
    ADD = mybir.AluOpType.add
    dt = mybir.dt.float32
    bf = mybir.dt.bfloat16

    nc = bacc.Bacc(
        "TRN2", target_bir_lowering=False, debug=False, num_devices=N_CORES
    )

    xe_wte = nc.declare_dram_parameter("xe_wte", [128, 16], dt, isOutput=False)
    xe_wpe = nc.declare_dram_parameter("xe_wpe", [128, 16], dt, isOutput=False)
    wqkv_r = nc.declare_dram_parameter("wqkv_r", [128, 16 * 768], bf, isOutput=False)
    keys_r = nc.declare_dram_parameter("keys_r", [128, 2 * 8192], bf, isOutput=False)
    vals_r = nc.declare_dram_parameter("vals_r", [128, 64 * 256], bf, isOutput=False)
    wo_r = nc.declare_dram_parameter("wo_r", [128, 2 * 2048], bf, isOutput=False)
    w1_r = nc.declare_dram_parameter("w1_r", [128, 16 * 1024], bf, isOutput=False)
    w2_r = nc.declare_dram_parameter("w2_r", [128, 8 * 2048], bf, isOutput=False)
    lm_r = nc.declare_dram_parameter("lm_r", [128, 16 * VPC], bf, isOutput=False)
    logits_out = nc.declare_dram_parameter("logits", [1, VPC], dt, isOutput=True)

    with tile.TileContext(nc) as tc:
        with (
            tc.tile_pool(name="const", bufs=1) as const,
            tc.tile_pool(name="small", bufs=1) as small,
            tc.tile_pool(name="stage", bufs=2) as stage,
            tc.tile_pool(name="ps", bufs=4, space="PSUM") as ps,
            tc.tile_pool(name="dram", bufs=1, space="DRAM") as dram,
            tc.tile_pool(name="stream", bufs=14) as stream,
        ):
            _snum = [0]

            def stile(label, width):
                _snum[0] += 1
                return stream.tile(
                    [128, width], bf, tag="s", name=f"s{_snum[0]}_{label}"
                )

            # Warm up the collectives path first: CC mesh init (~67us) starts
            # at the first collective trigger, and the first AllReduce runs
            # ~2x slower than later ones, so pay both early. The result is
            # consumed (x0) at the output tail only.
            warm_row = small.tile([128, 16], dt, tag="warm_row")
            nc.vector.memset(warm_row[:], 0.0)
            warm_in = dram.tile([128, 16], dt, tag="warm_in")
            warm_out = dram.tile([128, 16], dt, tag="warm_out")
            nc.scalar.dma_start(warm_in[:], warm_row[:])
            nc.gpsimd.collective_compute(
                "AllReduce",
                ADD,
                replica_groups=[list(range(N_CORES))],
                ins=[warm_in.opt()],
                outs=[warm_out.opt()],
            )
            warm_back = stage.tile([1, 16], dt, tag="warmb", bufs=1)
            nc.gpsimd.dma_start(warm_back[:], warm_out[0:1, :])

            ones_col = const.tile([128, 1], dt)
            nc.vector.memset(ones_col[:], 1.0)
            ones_row = const.tile([1, 128], dt)
            nc.vector.memset(ones_row[:], 1.0)
            one1 = const.tile([1, 1], dt)
            nc.vector.memset(one1[:], 1.0)
            eps_c = const.tile([1, 1], dt)
            nc.vector.memset(eps_c[:], EPS)
            junk = small.tile([1, 1], dt, tag="junk")
            # preload the ACT Sqrt LUT off the critical path
            nc.scalar.sqrt(junk[:], eps_c[:])

            def rms(xt, name, out_bf=False):
                """x * rsqrt(mean(x^2) + eps) for x in [128, 16] column layout."""
                sq = small.tile([128, 16], dt, tag=f"sq_{name}")
                ssum = small.tile([128, 1], dt, tag=f"ss_{name}")
                nc.scalar.activation(sq[:], xt[:], AF.Square, accum_out=ssum[:])
                tot = ps.tile([1, 1], dt, tag="b")
                nc.tensor.matmul(tot[:], ssum[:], ones_col[:], start=True, stop=True)
                std = small.tile([1, 1], dt, tag=f"std_{name}")
                nc.scalar.activation(
                    std[:], tot[:], AF.Sqrt, bias=eps_c[:], scale=1.0 / float(E)
                )
                inv = small.tile([1, 1], dt, tag=f"inv_{name}")
                nc.vector.reciprocal(inv[:], std[:])
                invb_ps = ps.tile([128, 1], dt, tag="b")
                nc.tensor.matmul(
                    invb_ps[:], ones_row[:], inv[:], start=True, stop=True
                )
                xn = small.tile([128, 16], bf if out_bf else dt, tag=f"xn_{name}")
                nc.vector.tensor_scalar_mul(xn[:], xt[:], invb_ps[:])
                return xn

            def to_bf(xt, name, w=16):
                xb = small.tile([128, w], bf, tag=f"bf_{name}")
                nc.vector.tensor_copy(xb[:], xt[:])
                return xb

            def row_to_col(row_sb, nblk, name):
                """[1, nblk*128] row (on one partition) -> PSUM [128, nblk]
                columns via nblk K=1 matmuls (no DMA round-trip)."""
                out = ps.tile([128, nblk], dt, tag="b", name=f"r2c_{name}")
                for c in range(nblk):
                    nc.tensor.matmul(
                        out[:, c : c + 1],
                        row_sb[0:1, c * 128 : (c + 1) * 128],
                        one1[:],
                        start=True, stop=True,
                    )
                return out

            # ---- embedding + double rms ----
            xw = stage.tile([128, 16], dt, tag="xw")
            nc.scalar.dma_start(xw[:], xe_wte[:])
            xp = stage.tile([128, 16], dt, tag="xp")
            nc.scalar.dma_start(xp[:], xe_wpe[:])
            x0 = small.tile([128, 16], dt, tag="x0")
            nc.vector.tensor_add(x0[:], xw[:], xp[:])
            x1 = rms(x0, "n1")  # residual input
            x2b = rms(x1, "n2", out_bf=True)

            # ---- qkv projection: [1, 768] row (q 0:256 | k 256:512 | v 512:768)
            # SCALE is folded into wq on the host. Two col-tiled chains
            # (rows 0 and 32 of one PSUM bank) run concurrently on PE.
            qkv_ps = ps.tile([128, 512], dt, tag="b", name="qkv_ps")
            for t in range(4):
                wt = stile("qkv", 3072)
                nc.sync.dma_start(wt[:], wqkv_r[:, t * 3072 : (t + 1) * 3072])
                for b in range(4):
                    k = 4 * t + b
                    nc.tensor.matmul(
                        qkv_ps[0:1, 0:512], x2b[:, k : k + 1],
                        wt[:, b * 768 : b * 768 + 512],
                        start=(k == 0), stop=(k == 15),
                    )
                    nc.tensor.matmul(
                        qkv_ps[32:33, 0:256], x2b[:, k : k + 1],
                        wt[:, b * 768 + 512 : b * 768 + 768],
                        start=(k == 0), stop=(k == 15),
                        tile_position=(0, 32),
                    )
            qkv_row = small.tile([1, 768], dt, tag="qkv")
            nc.scalar.copy(qkv_row[:, 0:512], qkv_ps[0:1, 0:512])
            nc.scalar.copy(qkv_row[:, 512:768], qkv_ps[32:33, 0:256])

            # ---- q,k to column layout via K=1 matmuls ----
            qkT_ps = row_to_col(qkv_row, 4, "qk")
            qkTb = small.tile([128, 4], bf, tag="qkTb")
            nc.vector.tensor_copy(qkTb[:], qkT_ps[:])

            # ---- attention scores in column layout: per head h and 128-row
            # t-block c, matmul(out[128,1], lhsT=K_block[128d,128t], rhs=q[128,1])
            # gives scores for t in [c*128, (c+1)*128) on partitions. ACT exp
            # straight from PSUM, accum_out accumulates softmax denominators.
            wTs = []
            esp = small.tile([128, 4], dt, tag="esp")  # per-partition exp sums
            for h in range(HPC):
                wTs.append(small.tile([128, 64], bf, tag=f"wT{h}", name=f"wT{h}"))
            for h in range(HPC):
                for j in range(2):
                    kt = stile("key", 4096)
                    nc.sync.dma_start(
                        kt[:],
                        keys_r[:, h * 8192 + j * 4096 : h * 8192 + (j + 1) * 4096],
                    )
                    qk_ps = ps.tile([128, 32], dt, tag="b", name=f"qk{h}_{j}")
                    for c in range(32):
                        nc.tensor.matmul(
                            qk_ps[:, c : c + 1],
                            kt[:, c * 128 : (c + 1) * 128],
                            qkTb[:, h : h + 1],
                            start=True, stop=True,
                        )
                    nc.scalar.activation(
                        wTs[h][:, j * 32 : (j + 1) * 32], qk_ps[:], AF.Exp,
                        accum_out=esp[:, h * 2 + j : h * 2 + j + 1],
                    )

            # current-token score per head: exp(q_h . k_h) (SCALE folded into q)
            e_last = small.tile([1, 2], dt, tag="elast")
            for h in range(HPC):
                pal = ps.tile([1, 1], dt, tag="b")
                nc.tensor.matmul(
                    pal[:], qkTb[:, h : h + 1], qkTb[:, 2 + h : 3 + h],
                    start=True, stop=True,
                )
                nc.scalar.activation(e_last[:, h : h + 1], pal[:], AF.Exp)

            # softmax denominators: cross-partition sum of esp + e_last
            dps = ps.tile([1, 4], dt, tag="b")
            nc.tensor.matmul(dps[:], ones_col[:], esp[:], start=True, stop=True)
            dtmp = small.tile([1, 2], dt, tag="dtmp")
            for h in range(HPC):
                nc.vector.reduce_sum(
                    dtmp[:, h : h + 1], dps[:, h * 2 : (h + 1) * 2],
                    axis=mybir.AxisListType.X,
                )
            nc.vector.tensor_add(dtmp[:], dtmp[:], e_last[:])
            dinv = small.tile([1, 2], dt, tag="dinv")
            nc.vector.reciprocal(dinv[:], dtmp[:])

            # ---- PV on PE: x_attn_h = sum_t w[t] * V[t, :] (unnormalized).
            # Per t-block c: matmul(out[1,128], lhsT=wT[:, c], rhs=V_block),
            # accumulating over all 64 blocks; the two heads run in different
            # PE column groups (rows 0 / 32 of one bank).
            pv_ps = ps.tile([128, 128], dt, tag="b", name="pv_ps")
            for tt in range(4):
                vt = stile("val", 4096)
                nc.sync.dma_start(vt[:], vals_r[:, tt * 4096 : (tt + 1) * 4096])
                for b in range(16):
                    c = tt * 16 + b
                    for h in range(HPC):
                        nc.tensor.matmul(
                            pv_ps[32 * h : 32 * h + 1, :],
                            wTs[h][:, c : c + 1],
                            vt[:, b * 256 + h * 128 : b * 256 + (h + 1) * 128],
                            start=(c == 0), stop=(c == 63),
                            tile_position=(0, 32 * h),
                        )

            # combine with current-token value, then normalize by the softmax sum
            xa_row = small.tile([1, 256], dt, tag="xa")
            for h in range(HPC):
                sl = slice(h * 128, (h + 1) * 128)
                nc.vector.tensor_scalar_mul(
                    xa_row[:, sl],
                    qkv_row[:, 512 + h * 128 : 512 + (h + 1) * 128],
                    e_last[:, h : h + 1],
                )
                nc.vector.tensor_add(
                    xa_row[:, sl], xa_row[:, sl], pv_ps[32 * h : 32 * h + 1, :]
                )
                nc.vector.tensor_scalar_mul(xa_row[:, sl], xa_row[:, sl], dinv[:, h : h + 1])

            xaT_ps = row_to_col(xa_row, 2, "xa")
            xaTb = small.tile([128, 2], bf, tag="xaTb")
            nc.vector.tensor_copy(xaTb[:], xaT_ps[:])

            # ---- wo partial, computed directly in column layout [128, 16]:
            # per output-block c, lhsT = wo block [128k, 128e], rhs = x_attn
            # column. The AllReduce then runs on a column-major DRAM layout so
            # the post-AR readback is a single [128, 16] DMA (both sides of an
            # elementwise AllReduce may use any fixed permutation).
            x3p_ps = ps.tile([128, 16], dt, tag="b", name="x3p_ps")
            wot = stile("wo", 4096)
            nc.sync.dma_start(wot[:], wo_r[:])
            for k in range(2):
                for c in range(16):
                    nc.tensor.matmul(
                        x3p_ps[:, c : c + 1],
                        wot[:, k * 2048 + c * 128 : k * 2048 + (c + 1) * 128],
                        xaTb[:, k : k + 1],
                        start=(k == 0), stop=(k == 1),
                    )
            x3p_sb = small.tile([128, 16], dt, tag="x3p")
            nc.vector.tensor_copy(x3p_sb[:], x3p_ps[:])
            ar1_d = dram.tile([128, 16], dt, tag="ar1_in_d")
            nc.scalar.dma_start(ar1_d[:], x3p_sb[:])

            def all_reduce(in_d, name):
                """Sum a [128, 16] column-major DRAM partial across cores and
                read it back; returns SBUF [128, 16]."""
                out_d = dram.tile([128, 16], dt, tag=f"{name}_out")
                nc.gpsimd.collective_compute(
                    "AllReduce",
                    ADD,
                    replica_groups=[list(range(N_CORES))],
                    ins=[in_d.opt()],
                    outs=[out_d.opt()],
                )
                rb = stage.tile([128, 16], dt, tag="rb", name=f"rb_{name}", bufs=2)
                nc.scalar.dma_start(rb[:], out_d[:])
                return rb

            x3_rb = all_reduce(ar1_d, "ar1")
            # re-preload the Sqrt LUT while the AllReduce is in flight
            nc.scalar.sqrt(junk[:], eps_c[:])
            x3 = small.tile([128, 16], dt, tag="x3")
            nc.vector.tensor_add(x3[:], x3_rb[:], x1[:])  # + residual

            # ---- MLP1: h = relu(w1 @ x4) via 2 col-tiled PE chains ----
            x4b = rms(x3, "n3", out_bf=True)
            mh_ps = ps.tile([128, 512], dt, tag="b", name="mh_ps")
            for a in range(4):
                w1t = stile("w1", 4096)
                nc.sync.dma_start(w1t[:], w1_r[:, a * 4096 : (a + 1) * 4096])
                for b in range(4):
                    k = a * 4 + b
                    for n in range(2):
                        nc.tensor.matmul(
                            mh_ps[32 * n : 32 * n + 1, :], x4b[:, k : k + 1],
                            w1t[:, b * 1024 + n * 512 : b * 1024 + (n + 1) * 512],
                            start=(k == 0), stop=(k == 15),
                            tile_position=(0, 32 * n),
                        )
            h_row = small.tile([1, 1024], dt, tag="hrow")
            for n in range(2):
                nc.scalar.activation(
                    h_row[:, n * 512 : (n + 1) * 512],
                    mh_ps[32 * n : 32 * n + 1, :], AF.Relu,
                )

            hT_ps = row_to_col(h_row, 8, "h")
            hTb = small.tile([128, 8], bf, tag="hTb")
            nc.vector.tensor_copy(hTb[:], hT_ps[:])

            # ---- MLP2: [1, 2048] partial via 4 col-tiled PE chains ----
            m2_ps = ps.tile([128, 512], dt, tag="b", name="m2_ps")
            for a in range(4):
                w2t = stile("w2", 4096)
                nc.sync.dma_start(w2t[:], w2_r[:, a * 4096 : (a + 1) * 4096])
                for b in range(2):
                    k = a * 2 + b
                    for n in range(4):
                        nc.tensor.matmul(
                            m2_ps[32 * n : 32 * n + 1, :], hTb[:, k : k + 1],
                            w2t[:, b * 2048 + n * 512 : b * 2048 + (n + 1) * 512],
                            start=(k == 0), stop=(k == 7),
                            tile_position=(0, 32 * n),
                        )
            m2sb = small.tile([128, 512], dt, tag="m2sb")
            nc.vector.tensor_copy(m2sb[:], m2_ps[:])
            m2c_ps = ps.tile([128, 16], dt, tag="b", name="m2c_ps")
            for r in range(4):
                for j in range(4):
                    nc.tensor.matmul(
                        m2c_ps[:, 4 * r + j : 4 * r + j + 1],
                        m2sb[32 * r : 32 * r + 1, j * 128 : (j + 1) * 128],
                        ones_col[32 * r : 32 * r + 1, :],
                        start=True, stop=True,
                        tile_position=(32 * r, 0),
                    )
            m2c_sb = small.tile([128, 16], dt, tag="m2c")
            nc.vector.tensor_copy(m2c_sb[:], m2c_ps[:])
            ar2_d = dram.tile([128, 16], dt, tag="ar2_in_d")
            nc.scalar.dma_start(ar2_d[:], m2c_sb[:])

            x5_rb = all_reduce(ar2_d, "ar2")
            x5b = small.tile([128, 16], bf, tag="x5b")
            nc.vector.tensor_add(x5b[:], x5_rb[:], x3[:])  # + residual, cast bf16

            # ---- LM head over the vocab shard: a single k-loop of full-width
            # [128, 6283] tiles. 13 output chunks as col-tiled PE chains, 4 per
            # PSUM bank (partition rows 0/32/64/96), all accumulating over the
            # 16 k-blocks concurrently.
            NCH = (VPC + 511) // 512  # 13
            banks = [
                ps.tile([128, 512], dt, tag="lmb", name=f"lmb{b}", bufs=4)
                for b in range((NCH + 3) // 4)
            ]

            def chain_out(c, cw):
                return banks[c // 4][32 * (c % 4) : 32 * (c % 4) + 1, 0:cw]

            for k in range(16):
                lt = stile("lm", VPC)
                nc.sync.dma_start(lt[:], lm_r[:, k * VPC : (k + 1) * VPC])
                for c in range(NCH):
                    cw = min(512, VPC - c * 512)
                    nc.tensor.matmul(
                        chain_out(c, cw), x5b[:, k : k + 1],
                        lt[:, c * 512 : c * 512 + cw],
                        start=(k == 0), stop=(k == 15),
                        tile_position=(0, 32 * (c % 4)),
                    )
            for b in range((NCH + 3) // 4):
                nch_b = min(4, NCH - 4 * b)  # chunks in this bank
                wid = VPC - 4 * b * 512 if nch_b < 4 else 2048
                ldr = small.tile([128, 512], dt, tag="ldr", name=f"ldr{b}", bufs=2)
                eng = nc.vector.tensor_copy if b % 2 == 0 else nc.scalar.copy
                eng(ldr[:, :], banks[b][:, :])
                if b == 0:
                    # keep the warmup-collective result alive off the
                    # critical path: ldr[0, :16] += 0 * warm
                    nc.vector.scalar_tensor_tensor(
                        ldr[0:1, 0:16], warm_back[:], 0.0, ldr[0:1, 0:16],
                        op0=MUL, op1=ADD,
                    )
                if nch_b == 4:
                    nc.scalar.dma_start(
                        logits_out[:, b * 2048 : b * 2048 + 2048],
                        ldr[0:128:32, :],
                    )
                else:
                    for r in range(nch_b):
                        c = 4 * b + r
                        cw = min(512, VPC - c * 512)
                        nc.scalar.dma_start(
                            logits_out[:, c * 512 : c * 512 + cw],
                            ldr[32 * r : 32 * r + 1, 0:cw],
                        )

    nc.finalize()
    return nc


def _col16(v):
    """[2048] vector -> [128, 16] column-major layout (e = c*128 + p at [p, c])."""
    return np.ascontiguousarray(v.reshape(16, 128).T)


def _part_major(mT, nblk, blk_rows, width):
    """[nblk*blk_rows, width] -> [blk_rows, nblk*width] partition-major, bf16."""
    out = mT.reshape(nblk, blk_rows, width).transpose(1, 0, 2).reshape(
        blk_rows, nblk * width
    )
    return np.ascontiguousarray(out.astype(ml_dtypes.bfloat16))


def _prep_in_maps(token_id, pos_id, keys, values, wte, wpe, wq, wk, wv, wo, w1, w2, lm_w):
    f32 = lambda a: np.asarray(a, dtype=np.float32)
    keys, values = f32(keys), f32(values)
    wq, wk, wv, wo, w1, w2, lm_w = map(f32, (wq, wk, wv, wo, w1, w2, lm_w))
    wq = wq * np.float32(SCALE)  # fold attention scale into q
    xe_wte = _col16(f32(wte[token_id]))
    xe_wpe = _col16(f32(wpe[pos_id]))
    lm_pad = np.zeros((N_CORES * VPC, E), np.float32)
    lm_pad[:VOCAB] = lm_w

    in_maps = []
    for i in range(N_CORES):
        hs = slice(i * EPC, (i + 1) * EPC)
        wqkv = np.concatenate([wq[hs], wk[hs], wv[hs]], axis=0)  # [768, E]
        in_maps.append(
            {
                "xe_wte": xe_wte,
                "xe_wpe": xe_wpe,
                "wqkv_r": _part_major(np.ascontiguousarray(wqkv.T), 16, 128, 768),
                "keys_r": _part_major(np.ascontiguousarray(keys[:, hs].T), 2, 128, 8192),
                "vals_r": _part_major(values[:, hs], 64, 128, EPC),
                "wo_r": _part_major(np.ascontiguousarray(wo[:, hs].T), 2, 128, E),
                "w1_r": _part_major(
                    np.ascontiguousarray(w1[i * 1024 : (i + 1) * 1024].T), 16, 128, 1024
                ),
                "w2_r": _part_major(
                    np.ascontiguousarray(w2[:, i * 1024 : (i + 1) * 1024].T), 8, 128, E
                ),
                "lm_r": _part_major(
                    np.ascontiguousarray(lm_pad[i * VPC : (i + 1) * VPC].T), 16, 128, VPC
                ),
            }
        )
    return in_maps


def kernel(**inputs) -> np.ndarray:
    from concourse.bass_utils import run_bass_kernel_spmd

    token_id = int(inputs["token_id"])
    pos_id = int(inputs["pos_id"])
    in_maps = _prep_in_maps(
        token_id,
        pos_id,
        inputs["keys"],
        inputs["values"],
        inputs["wte"],
        inputs["wpe"],
        inputs["wq"],
        inputs["wk"],
        inputs["wv"],
        inputs["wo"],
        inputs["w1"],
        inputs["w2"],
        inputs["lm_w"],
    )
    if "nc" not in _CACHE:
        _CACHE["nc"] = _build_nc()
    nc = _CACHE["nc"]
    res = run_bass_kernel_spmd(
        nc,
        in_maps,
        core_ids=list(range(N_CORES)),
        trace=TRACE,
        trace_cores=[0] if TRACE else None,
    )
    _CACHE["last_result"] = res
    logits = np.concatenate([r["logits"][0] for r in res.results])[:VOCAB]
    return np.ascontiguousarray(logits.astype(np.float32))


# revision 16
# speedup vs baseline: 1.4687x; 1.4687x over previous
"""Tensor-parallel MiniGPT single-token decode step on 8 Trainium2 NeuronCores.

Sharding (per core i of 8):
  - attention: heads 2i, 2i+1 (head_dim 128 -> cols i*256:(i+1)*256 of E=2048);
    wq/wk/wv row-sharded, wo column-sharded, KV cache column-sharded by head.
  - MLP: w1 row-sharded (1024 rows/core), w2 column-sharded.
  - LM head: vocab-sharded (50257 padded to 8*6283=50264 rows).
  - Two 8KB AllReduces combine the wo- and w2- partial sums; logits are
    gathered on the host.

All weights are cast to bf16 on the host and laid out as [128, F]
partition-major arrays, so every device DMA is one contiguous ~1-1.6MB run at
half the fp32 byte count. Activations stay fp32; PE matmuls take bf16 operands
and accumulate fp32 in PSUM.

Matvec strategy: PE does nearly everything. x-chunks are the [128, 1]
stationary operand, weight tiles stream as the moving operand in N<=512 chunks
with PSUM accumulation across k-blocks. Output chunks are spread across PSUM
*partition rows* 0/32/64/96 (PE column tiling via out.base_partition), so up
to 4 chains run concurrently in different column groups of the array and 4
chains share one PSUM bank. Attention scores are computed directly in column
layout (key-block stationary, q moving); exp runs on ACT straight from PSUM
with accum_out providing softmax denominators. Row->column transposes use K=1
matmuls (lhsT = the row, rhs = a [1,1] one) instead of DMA reshapes. The
collective path is warmed with an early AllReduce whose result is consumed at
the output tail.
"""

import numpy as np
import ml_dtypes

N_CORES = 8
E = 2048
HPC = 2  # heads per core
EPC = HPC * 128  # 256
T = 8192
VOCAB = 50257
VPC = 6283  # padded vocab rows per core (8 * 6283 = 50264)
SCALE = float(1.0 / np.sqrt(128.0))
EPS = 1e-5

_CACHE = {}
TRACE = False


def _build_nc():
    import concourse.bacc as bacc
    import concourse.mybir as mybir
    import concourse.tile as tile

    AF = mybir.ActivationFunctionType
    MUL = mybir.AluOpType.mult
    ADD = mybir.AluOpType.add
    dt = mybir.dt.float32
    bf = mybir.dt.bfloat16

    nc = bacc.Bacc(
        "TRN2", target_bir_lowering=False, debug=False, num_devices=N_CORES
    )

    xe_wte = nc.declare_dram_parameter("xe_wte", [128, 16], dt, isOutput=False)
    xe_wpe = nc.declare_dram_parameter("xe_wpe", [128, 16], dt, isOutput=False)
    wqkv_r = nc.declare_dram_parameter("wqkv_r", [128, 16 * 768], bf, isOutput=False)
    keys_r = nc.declare_dram_parameter("keys_r", [128, 2 * 8192], bf, isOutput=False)
    vals_r = nc.declare_dram_parameter("vals_r", [128, 64 * 256], bf, isOutput=False)
    wo_r = nc.declare_dram_parameter("wo_r", [128, 2 * 2048], bf, isOutput=False)
    w1_r = nc.declare_dram_parameter("w1_r", [128, 16 * 1024], bf, isOutput=False)
    w2_r = nc.declare_dram_parameter("w2_r", [128, 8 * 2048], bf, isOutput=False)
    lm_r = nc.declare_dram_parameter("lm_r", [128, 16 * VPC], bf, isOutput=False)
    logits_out = nc.declare_dram_parameter("logits", [1, VPC], dt, isOutput=True)

    with tile.TileContext(nc) as tc:
        with (
            tc.tile_pool(name="const", bufs=1) as const,
            tc.tile_pool(name="small", bufs=1) as small,
            tc.tile_pool(name="stage", bufs=2) as stage,
            tc.tile_pool(name="ps", bufs=4, space="PSUM") as ps,
            tc.tile_pool(name="dram", bufs=1, space="DRAM") as dram,
            tc.tile_pool(name="stream", bufs=14) as stream,
        ):
            _snum = [0]

            def stile(label, width):
                _snum[0] += 1
                return stream.tile(
                    [128, width], bf, tag="s", name=f"s{_snum[0]}_{label}"
                )

            # Warm up the collectives path first: CC mesh init (~67us) starts
            # at the first collective trigger, and the first AllReduce runs
            # ~2x slower than later ones, so pay both early. The result is
            # consumed (x0) at the output tail only.
            warm_row = small.tile([128, 16], dt, tag="warm_row")
            nc.vector.memset(warm_row[:], 0.0)
            warm_in = dram.tile([128, 16], dt, tag="warm_in")
            warm_out = dram.tile([128, 16], dt, tag="warm_out")
            nc.scalar.dma_start(warm_in[:], warm_row[:])
            nc.gpsimd.collective_compute(
                "AllReduce",
                ADD,
                replica_groups=[list(range(N_CORES))],
                ins=[warm_in.opt()],
                outs=[warm_out.opt()],
            )
            warm_back = stage.tile([1, 16], dt, tag="warmb", bufs=1)
            nc.gpsimd.dma_start(warm_back[:], warm_out[0:1, :])

            ones_col = const.tile([128, 1], dt)
            nc.vector.memset(ones_col[:], 1.0)
            ones_row = const.tile([1, 128], dt)
            nc.vector.memset(ones_row[:], 1.0)
            one1 = const.tile([1, 1], dt)
            nc.vector.memset(one1[:], 1.0)
            eps_c = const.tile([1, 1], dt)
            nc.vector.memset(eps_c[:], EPS)
            junk = small.tile([1, 1], dt, tag="junk")
            # preload the ACT Sqrt LUT off the critical path
            nc.scalar.sqrt(junk[:], eps_c[:])

            def rms(xt, name, out_bf=False):
                """x * rsqrt(mean(x^2) + eps) for x in [128, 16] column layout."""
                sq = small.tile([128, 16], dt, tag=f"sq_{name}")
                ssum = small.tile([128, 1], dt, tag=f"ss_{name}")
                nc.scalar.activation(sq[:], xt[:], AF.Square, accum_out=ssum[:])
                tot = ps.tile([1, 1], dt, tag="b")
                nc.tensor.matmul(tot[:], ssum[:], ones_col[:], start=True, stop=True)
                std = small.tile([1, 1], dt, tag=f"std_{name}")
                nc.scalar.activation(
                    std[:], tot[:], AF.Sqrt, bias=eps_c[:], scale=1.0 / float(E)
                )
                inv = small.tile([1, 1], dt, tag=f"inv_{name}")
                nc.vector.reciprocal(inv[:], std[:])
                invb_ps = ps.tile([128, 1], dt, tag="b")
                nc.tensor.matmul(
                    invb_ps[:], ones_row[:], inv[:], start=True, stop=True
                )
                xn = small.tile([128, 16], bf if out_bf else dt, tag=f"xn_{name}")
                nc.vector.tensor_scalar_mul(xn[:], xt[:], invb_ps[:])
                return xn

            def to_bf(xt, name, w=16):
                xb = small.tile([128, w], bf, tag=f"bf_{name}")
                nc.vector.tensor_copy(xb[:], xt[:])
                return xb

            def row_to_col(row_sb, nblk, name):
                """[1, nblk*128] row (on one partition) -> PSUM [128, nblk]
                columns via nblk K=1 matmuls (no DMA round-trip)."""
                out = ps.tile([128, nblk], dt, tag="b", name=f"r2c_{name}")
                for c in range(nblk):
                    nc.tensor.matmul(
                        out[:, c : c + 1],
                        row_sb[0:1, c * 128 : (c + 1) * 128],
                        one1[:],
                        start=True, stop=True,
                    )
                return out

            # ---- embedding + double rms ----
            xw = stage.tile([128, 16], dt, tag="xw")
            nc.scalar.dma_start(xw[:], xe_wte[:])
            xp = stage.tile([128, 16], dt, tag="xp")
            nc.scalar.dma_start(xp[:], xe_wpe[:])
            x0 = small.tile([128, 16], dt, tag="x0")
            nc.vector.tensor_add(x0[:], xw[:], xp[:])
            x1 = rms(x0, "n1")  # residual input
            x2b = rms(x1, "n2", out_bf=True)

            # ---- qkv projection: [1, 768] row (q 0:256 | k 256:512 | v 512:768)
            # SCALE is folded into wq on the host. Two col-tiled chains
            # (rows 0 and 32 of one PSUM bank) run concurrently on PE.
            qkv_ps = ps.tile([128, 512], dt, tag="b", name="qkv_ps")
            for t in range(4):
                wt = stile("qkv", 3072)
                nc.sync.dma_start(wt[:], wqkv_r[:, t * 3072 : (t + 1) * 3072])
                for b in range(4):
                    k = 4 * t + b
                    nc.tensor.matmul(
                        qkv_ps[0:1, 0:512], x2b[:, k : k + 1],
                        wt[:, b * 768 : b * 768 + 512],
                        start=(k == 0), stop=(k == 15),
                    )
                    nc.tensor.matmul(
                        qkv_ps[32:33, 0:256], x2b[:, k : k + 1],
                        wt[:, b * 768 + 512 : b * 768 + 768],
                        start=(k == 0), stop=(k == 15),
                        tile_position=(0, 32),
                    )
            qkv_row = small.tile([1, 768], dt, tag="qkv")
            nc.scalar.copy(qkv_row[:, 0:512], qkv_ps[0:1, 0:512])
            nc.scalar.copy(qkv_row[:, 512:768], qkv_ps[32:33, 0:256])

            # ---- q,k to column layout via K=1 matmuls ----
            qkT_ps = row_to_col(qkv_row, 4, "qk")
            qkTb = small.tile([128, 4], bf, tag="qkTb")
            nc.vector.tensor_copy(qkTb[:], qkT_ps[:])

            # ---- attention scores in column layout: per head h and 128-row
            # t-block c, matmul(out[128,1], lhsT=K_block[128d,128t], rhs=q[128,1])
            # gives scores for t in [c*128, (c+1)*128) on partitions. ACT exp
            # straight from PSUM, accum_out accumulates softmax denominators.
            wTs = []
            esp = small.tile([128, 4], dt, tag="esp")  # per-partition exp sums
            for h in range(HPC):
                wTs.append(small.tile([128, 64], bf, tag=f"wT{h}", name=f"wT{h}"))
            for h in range(HPC):
                for j in range(2):
                    kt = stile("key", 4096)
                    nc.sync.dma_start(
                        kt[:],
                        keys_r[:, h * 8192 + j * 4096 : h * 8192 + (j + 1) * 4096],
                    )
                    qk_ps = ps.tile([128, 32], dt, tag="b", name=f"qk{h}_{j}")
                    for c in range(32):
                        nc.tensor.matmul(
                            qk_ps[:, c : c + 1],
                            kt[:, c * 128 : (c + 1) * 128],
                            qkTb[:, h : h + 1],
                            start=True, stop=True,
                        )
                    nc.scalar.activation(
                        wTs[h][:, j * 32 : (j + 1) * 32], qk_ps[:], AF.Exp,
                        accum_out=esp[:, h * 2 + j : h * 2 + j + 1],
                    )

            # current-token score per head: exp(q_h . k_h) (SCALE folded into q)
            e_last = small.tile([1, 2], dt, tag="elast")
            for h in range(HPC):
                pal = ps.tile([1, 1], dt, tag="b")
                nc.tensor.matmul(
                    pal[:], qkTb[:, h : h + 1], qkTb[:, 2 + h : 3 + h],
                    start=True, stop=True,
                )
                nc.scalar.activation(e_last[:, h : h + 1], pal[:], AF.Exp)

            # softmax denominators: cross-partition sum of esp + e_last
            dps = ps.tile([1, 4], dt, tag="b")
            nc.tensor.matmul(dps[:], ones_col[:], esp[:], start=True, stop=True)
            dtmp = small.tile([1, 2], dt, tag="dtmp")
            for h in range(HPC):
                nc.vector.reduce_sum(
                    dtmp[:, h : h + 1], dps[:, h * 2 : (h + 1) * 2],
                    axis=mybir.AxisListType.X,
                )
            nc.vector.tensor_add(dtmp[:], dtmp[:], e_last[:])
            dinv = small.tile([1, 2], dt, tag="dinv")
            nc.vector.reciprocal(dinv[:], dtmp[:])

            # ---- PV on PE: x_attn_h = sum_t w[t] * V[t, :] (unnormalized).
            # Per t-block c: matmul(out[1,128], lhsT=wT[:, c], rhs=V_block),
            # accumulating over all 64 blocks; the two heads run in different
            # PE column groups (rows 0 / 32 of one bank).
            pv_ps = ps.tile([128, 128], dt, tag="b", name="pv_ps")
            for tt in range(4):
                vt = stile("val", 4096)
                nc.sync.dma_start(vt[:], vals_r[:, tt * 4096 : (tt + 1) * 4096])
                for b in range(16):
                    c = tt * 16 + b
                    for h in range(HPC):
                        nc.tensor.matmul(
                            pv_ps[32 * h : 32 * h + 1, :],
                            wTs[h][:, c : c + 1],
                            vt[:, b * 256 + h * 128 : b * 256 + (h + 1) * 128],
                            start=(c == 0), stop=(c == 63),
                            tile_position=(0, 32 * h),
                        )

            # combine with current-token value, then normalize by the softmax sum
            xa_row = small.tile([1, 256], dt, tag="xa")
            for h in range(HPC):
                sl = slice(h * 128, (h + 1) * 128)
                nc.vector.tensor_scalar_mul(
                    xa_row[:, sl],
                    qkv_row[:, 512 + h * 128 : 512 + (h + 1) * 128],
                    e_last[:, h : h + 1],
                )
                nc.vector.tensor_add(
                    xa_row[:, sl], xa_row[:, sl], pv_ps[32 * h : 32 * h + 1, :]
                )
                nc.vector.tensor_scalar_mul(xa_row[:, sl], xa_row[:, sl], dinv[:, h : h + 1])

            xaT_ps = row_to_col(xa_row, 2, "xa")
            xaTb = small.tile([128, 2], bf, tag="xaTb")
            nc.vector.tensor_copy(xaTb[:], xaT_ps[:])

            # ---- wo partial, computed directly in column layout [128, 16]:
            # per output-block c, lhsT = wo block [128k, 128e], rhs = x_attn
            # column. The AllReduce then runs on a column-major DRAM layout so
            # the post-AR readback is a single [128, 16] DMA (both sides of an
            # elementwise AllReduce may use any fixed permutation).
            x3p_ps = ps.tile([128, 16], dt, tag="b", name="x3p_ps")
            wot = stile("wo", 4096)
            nc.sync.dma_start(wot[:], wo_r[:])
            for k in range(2):
                for c in range(16):
                    nc.tensor.matmul(
                        x3p_ps[:, c : c + 1],
                        wot[:, k * 2048 + c * 128 : k * 2048 + (c + 1) * 128],
                        xaTb[:, k : k + 1],
                        start=(k == 0), stop=(k == 1),
                    )
            x3p_sb = small.tile([128, 16], dt, tag="x3p")
            nc.vector.tensor_copy(x3p_sb[:], x3p_ps[:])
            ar1_d = dram.tile([128, 16], dt, tag="ar1_in_d")
            nc.scalar.dma_start(ar1_d[:], x3p_sb[:])

            def all_reduce(in_d, name):
                """Sum a [128, 16] column-major DRAM partial across cores and
                read it back; returns SBUF [128, 16]."""
                out_d = dram.tile([128, 16], dt, tag=f"{name}_out")
                nc.gpsimd.collective_compute(
                    "AllReduce",
                    ADD,
                    replica_groups=[list(range(N_CORES))],
                    ins=[in_d.opt()],
                    outs=[out_d.opt()],
                )
                rb = stage.tile([128, 16], dt, tag="rb", name=f"rb_{name}", bufs=2)
                nc.scalar.dma_start(rb[:], out_d[:])
                return rb

            x3_rb = all_reduce(ar1_d, "ar1")
            # re-preload the Sqrt LUT while the AllReduce is in flight
            nc.scalar.sqrt(junk[:], eps_c[:])
            x3 = small.tile([128, 16], dt, tag="x3")
            nc.vector.tensor_add(x3[:], x3_rb[:], x1[:])  # + residual

            # ---- MLP1: h = relu(w1 @ x4) via 2 col-tiled PE chains ----
            x4b = rms(x3, "n3", out_bf=True)
            mh_ps = ps.tile([128, 512], dt, tag="b", name="mh_ps")
            for a in range(4):
                w1t = stile("w1", 4096)
                nc.sync.dma_start(w1t[:], w1_r[:, a * 4096 : (a + 1) * 4096])
                for b in range(4):
                    k = a * 4 + b
                    for n in range(2):
                        nc.tensor.matmul(
                            mh_ps[32 * n : 32 * n + 1, :], x4b[:, k : k + 1],
                            w1t[:, b * 1024 + n * 512 : b * 1024 + (n + 1) * 512],
                            start=(k == 0), stop=(k == 15),
                            tile_position=(0, 32 * n),
                        )
            h_row = small.tile([1, 1024], dt, tag="hrow")
            for n in range(2):
                nc.scalar.activation(
                    h_row[:, n * 512 : (n + 1) * 512],
                    mh_ps[32 * n : 32 * n + 1, :], AF.Relu,
                )

            hT_ps = row_to_col(h_row, 8, "h")
            hTb = small.tile([128, 8], bf, tag="hTb")
            nc.vector.tensor_copy(hTb[:], hT_ps[:])

            # ---- MLP2: [1, 2048] partial via 4 col-tiled PE chains ----
            m2_ps = ps.tile([128, 512], dt, tag="b", name="m2_ps")
            for a in range(4):
                w2t = stile("w2", 4096)
                nc.sync.dma_start(w2t[:], w2_r[:, a * 4096 : (a + 1) * 4096])
                for b in range(2):
                    k = a * 2 + b
                    for n in range(4):
                        nc.tensor.matmul(
                            m2_ps[32 * n : 32 * n + 1, :], hTb[:, k : k + 1],
                            w2t[:, b * 2048 + n * 512 : b * 2048 + (n + 1) * 512],
                            start=(k == 0), stop=(k == 7),
                            tile_position=(0, 32 * n),
                        )
            m2row = small.tile([1, 2048], dt, tag="m2row")
            for n in range(4):
                eng = nc.vector.tensor_copy if n % 2 == 0 else nc.scalar.copy
                eng(m2row[:, n * 512 : (n + 1) * 512], m2_ps[32 * n : 32 * n + 1, :])
            m2c_ps = row_to_col(m2row, 16, "m2c")
            m2c_sb = small.tile([128, 16], dt, tag="m2c")
            nc.vector.tensor_copy(m2c_sb[:], m2c_ps[:])
            ar2_d = dram.tile([128, 16], dt, tag="ar2_in_d")
            nc.scalar.dma_start(ar2_d[:], m2c_sb[:])

            x5_rb = all_reduce(ar2_d, "ar2")
            x5b = small.tile([128, 16], bf, tag="x5b")
            nc.vector.tensor_add(x5b[:], x5_rb[:], x3[:])  # + residual, cast bf16

            # ---- LM head over the vocab shard: a single k-loop of full-width
            # [128, 6283] tiles. 13 output chunks as col-tiled PE chains, 4 per
            # PSUM bank (partition rows 0/32/64/96), all accumulating over the
            # 16 k-blocks concurrently.
            NCH = (VPC + 511) // 512  # 13
            banks = [
                ps.tile([128, 512], dt, tag="lmb", name=f"lmb{b}", bufs=4)
                for b in range((NCH + 3) // 4)
            ]

            def chain_out(c, cw):
                return banks[c // 4][32 * (c % 4) : 32 * (c % 4) + 1, 0:cw]

            for k in range(16):
                lt = stile("lm", VPC)
                nc.sync.dma_start(lt[:], lm_r[:, k * VPC : (k + 1) * VPC])
                for c in range(NCH):
                    cw = min(512, VPC - c * 512)
                    nc.tensor.matmul(
                        chain_out(c, cw), x5b[:, k : k + 1],
                        lt[:, c * 512 : c * 512 + cw],
                        start=(k == 0), stop=(k == 15),
                        tile_position=(0, 32 * (c % 4)),
                    )
            for b in range((NCH + 3) // 4):
                nch_b = min(4, NCH - 4 * b)  # chunks in this bank
                wid = VPC - 4 * b * 512 if nch_b < 4 else 2048
                ldr = small.tile([128, 512], dt, tag="ldr", name=f"ldr{b}", bufs=2)
                eng = nc.vector.tensor_copy if b % 2 == 0 else nc.scalar.copy
                eng(ldr[:, :], banks[b][:, :])
                if b == 0:
                    # keep the warmup-collective result alive off the
                    # critical path: ldr[0, :16] += 0 * warm
                    nc.vector.scalar_tensor_tensor(
                        ldr[0:1, 0:16], warm_back[:], 0.0, ldr[0:1, 0:16],
                        op0=MUL, op1=ADD,
                    )
                if nch_b == 4:
                    nc.scalar.dma_start(
                        logits_out[:, b * 2048 : b * 2048 + 2048],
                        ldr[0:128:32, :],
                    )
                else:
                    for r in range(nch_b):
                        c = 4 * b + r
                        cw = min(512, VPC - c * 512)
                        nc.scalar.dma_start(
                            logits_out[:, c * 512 : c * 512 + cw],
                            ldr[32 * r : 32 * r + 1, 0:cw],
                        )

    nc.finalize()
    return nc


def _col16(v):
    """[2048] vector -> [128, 16] column-major layout (e = c*128 + p at [p, c])."""
    return np.ascontiguousarray(v.reshape(16, 128).T)


def _part_major(mT, nblk, blk_rows, width):
    """[nblk*blk_rows, width] -> [blk_rows, nblk*width] partition-major, bf16."""
    out = mT.reshape(nblk, blk_rows, width).transpose(1, 0, 2).reshape(
        blk_rows, nblk * width
    )
    return np.ascontiguousarray(out.astype(ml_dtypes.bfloat16))


def _prep_in_maps(token_id, pos_id, keys, values, wte, wpe, wq, wk, wv, wo, w1, w2, lm_w):
    f32 = lambda a: np.asarray(a, dtype=np.float32)
    keys, values = f32(keys), f32(values)
    wq, wk, wv, wo, w1, w2, lm_w = map(f32, (wq, wk, wv, wo, w1, w2, lm_w))
    wq = wq * np.float32(SCALE)  # fold attention scale into q
    xe_wte = _col16(f32(wte[token_id]))
    xe_wpe = _col16(f32(wpe[pos_id]))
    lm_pad = np.zeros((N_CORES * VPC, E), np.float32)
    lm_pad[:VOCAB] = lm_w

    in_maps = []
    for i in range(N_CORES):
        hs = slice(i * EPC, (i + 1) * EPC)
        wqkv = np.concatenate([wq[hs], wk[hs], wv[hs]], axis=0)  # [768, E]
        in_maps.append(
            {
                "xe_wte": xe_wte,
                "xe_wpe": xe_wpe,
                "wqkv_r": _part_major(np.ascontiguousarray(wqkv.T), 16, 128, 768),
                "keys_r": _part_major(np.ascontiguousarray(keys[:, hs].T), 2, 128, 8192),
                "vals_r": _part_major(values[:, hs], 64, 128, EPC),
                "wo_r": _part_major(np.ascontiguousarray(wo[:, hs].T), 2, 128, E),
                "w1_r": _part_major(
                    np.ascontiguousarray(w1[i * 1024 : (i + 1) * 1024].T), 16, 128, 1024
                ),
                "w2_r": _part_major(
                    np.ascontiguousarray(w2[:, i * 1024 : (i + 1) * 1024].T), 8, 128, E
                ),
                "lm_r": _part_major(
                    np.ascontiguousarray(lm_pad[i * VPC : (i + 1) * VPC].T), 16, 128, VPC
                ),
            }
        )
    return in_maps


def kernel(**inputs) -> np.ndarray:
    from concourse.bass_utils import run_bass_kernel_spmd

    token_id = int(inputs["token_id"])
    pos_id = int(inputs["pos_id"])
    in_maps = _prep_in_maps(
        token_id,
        pos_id,
        inputs["keys"],
        inputs["values"],
        inputs["wte"],
        inputs["wpe"],
        inputs["wq"],
        inputs["wk"],
        inputs["wv"],
        inputs["wo"],
        inputs["w1"],
        inputs["w2"],
        inputs["lm_w"],
    )
    if "nc" not in _CACHE:
        _CACHE["nc"] = _build_nc()
    nc = _CACHE["nc"]
    res = run_bass_kernel_spmd(
        nc,
        in_maps,
        core_ids=list(range(N_CORES)),
        trace=TRACE,
        trace_cores=[0] if TRACE else None,
    )
    _CACHE["last_result"] = res
    logits = np.concatenate([r["logits"][0] for r in res.results])[:VOCAB]
    return np.ascontiguousarray(logits.astype(np.float32))
